# revision 27
# baseline (speedup 1.0000x reference)
"""Contrastive loss kernel for Trainium2 (8 NeuronCores).

loss = mean((sim.sum(-1) - diag) / T) with sim = n @ n.T, n = x/||x||
     = (||s||^2 - N) / (N*T)          with s = sum_i x_i / ||x_i||

Each core takes a [2048, 512] row shard, shipped as fp16 packed
[128, 16, 512] (partition p holds rows 16p..16p+15), streamed in eight
2-segment DMAs so stats pipeline behind the transfers.  Per 512-dim
segment t the row sum-of-squares ss[:, t] is computed on DVE
(2x-mode tensor_mul square + 4x-mode tensor_scalar add-reduce) or on
ACT (Square activation with accum_out), interleaved to balance the two
engines.  rn = sqrt(1/ss) via DVE reciprocal + ACT sqrt; a dummy sqrt
on a const AP runs first so a single activation-table load (the
sqrt_and_others set, which also contains square) covers the kernel.
The weighted row sum s = sum_t x_t^T @ rn_t runs on the PE as 64 tiny
matmuls with the x tile as the 128x128 stationary operand and rn[:, t]
as the 1-column moving operand; chunk c of s accumulates in its own
PSUM bank (each accumulation group needs its own 2KB zero region).
The [128, 4] result is copied to SBUF (split across DVE/ACT) and DMA'd
out per core; the host sums the 8 partials and applies the scalar
epilogue.
"""

import numpy as np

import concourse.bass as bass
import concourse.bacc as bacc
import concourse.tile as tile
from concourse import mybir
from concourse.bass_utils import run_bass_kernel_spmd

N = 16384
D = 512
NCORES = 8
ROWS = N // NCORES    # 2048 rows per core
P = 128               # SBUF partitions
NSEG = ROWS // P      # 16 segments of [128, 512] per core
DCH = D // P          # 4 psum chunks of 128 dims
TEMPERATURE = 0.5

# DMA chunking: segments per input DMA (sum = NSEG)
DMA_CHUNKS = (2, 2, 2, 2, 2, 2, 2, 2)
# engine per segment: 'w' = DVE square+accum, 'a' = ACT square+accum
STATS_ENG = "wawawawawawawwaw"
# rsqrt batches (sum = NSEG)
RSQ_GROUPS = (12, 3, 1)

F32 = mybir.dt.float32
F16 = mybir.dt.float16
SQUARE = mybir.ActivationFunctionType.Square

_NC = None


def _build_nc(dma_chunks=None, stats_eng=None, rsq_groups=None) -> bass.Bass:
    dma_chunks = dma_chunks or DMA_CHUNKS
    stats_eng = stats_eng or STATS_ENG
    rsq_groups = rsq_groups or RSQ_GROUPS
    nc = bacc.Bacc(None)
    x_in = nc.declare_dram_parameter("x", [P, NSEG, D], F16, isOutput=False)
    s_out = nc.declare_dram_parameter("s", [P, DCH], F32, isOutput=True)

    with tile.TileContext(nc) as tc:
        with (
            tc.tile_pool(name="xs", bufs=1) as xs_pool,
            tc.tile_pool(name="scr", bufs=1) as scr_pool,
            tc.tile_pool(name="st", bufs=1) as st_pool,
            tc.tile_pool(name="acc", bufs=1, space="PSUM") as psum_pool,
        ):
            xt = xs_pool.tile([P, NSEG, D], F16, tag="x")
            scr_v = scr_pool.tile([P, D], F16, tag="scr_v")
            scr_a = scr_pool.tile([P, D], F16, tag="scr_a")
            scr_d = scr_pool.tile([P, D], F16, tag="scr_d")
            scr_p = scr_pool.tile([P, D], F16, tag="scr_p")
            ss = st_pool.tile([P, NSEG], F32, tag="ss")
            ri = st_pool.tile([P, NSEG], F32, tag="ri")
            rn = st_pool.tile([P, NSEG], F16, tag="rn")
            # one full PSUM bank (2KB zero region) per accumulation group
            acc0 = psum_pool.tile([P, 512], F32, tag="acc0")
            acc1 = psum_pool.tile([P, 512], F32, tag="acc1")
            acc2 = psum_pool.tile([P, 512], F32, tag="acc2")
            acc3 = psum_pool.tile([P, 512], F32, tag="acc3")
            accs = [acc0, acc1, acc2, acc3]
            res = st_pool.tile([P, DCH], F32, tag="res")
            dum = st_pool.tile([P, 1], F32, tag="dum")

            # Dummy sqrt first so the single activation table loaded covers
            # both Sqrt and Square (sqrt_and_others); runs under the DMA head.
            # Input is a const AP so the sqrt has no cross-engine deps.
            nc.scalar.sqrt(out=dum, in_=nc.const_aps.tensor(0.0, (P, 1)))

            base = 0
            for csz in dma_chunks:
                nc.sync.dma_start(
                    out=xt[:, base : base + csz, :],
                    in_=x_in[:, base : base + csz, :],
                )
                base += csz

            def emit_stats(t):
                if stats_eng[t] == "a":
                    nc.scalar.activation(
                        out=scr_a,
                        in_=xt[:, t, :],
                        func=SQUARE,
                        accum_out=ss[:, t : t + 1],
                    )
                elif stats_eng[t] == "p":
                    # Pool squares; DVE does the 4x-mode accumulate
                    nc.gpsimd.tensor_mul(scr_p, xt[:, t, :], xt[:, t, :])
                    nc.vector.tensor_scalar(
                        out=scr_d,
                        in0=scr_p,
                        scalar1=1.0,
                        scalar2=0.0,
                        op0=mybir.AluOpType.mult,
                        op1=mybir.AluOpType.add,
                        accum_out=ss[:, t : t + 1],
                    )
                else:
                    # DVE two-op: 2x-mode square then 4x-mode add-reduce
                    nc.vector.tensor_mul(scr_v, xt[:, t, :], xt[:, t, :])
                    nc.vector.tensor_scalar(
                        out=scr_d,
                        in0=scr_v,
                        scalar1=1.0,
                        scalar2=0.0,
                        op0=mybir.AluOpType.mult,
                        op1=mybir.AluOpType.add,
                        accum_out=ss[:, t : t + 1],
                    )

            base = 0
            for gsz in rsq_groups:
                lo, hi = base, base + gsz
                base += gsz
                for t in range(lo, hi):
                    emit_stats(t)
                nc.vector.reciprocal(out=ri[:, lo:hi], in_=ss[:, lo:hi])
                with nc.allow_low_precision(reason="fp16 rnorm for PE rhs"):
                    nc.scalar.sqrt(out=rn[:, lo:hi], in_=ri[:, lo:hi])
                for t in range(lo, hi):
                    for c in range(DCH):
                        nc.tensor.matmul(
                            accs[c][:, 0:1],
                            lhsT=xt[:, t, c * P : (c + 1) * P],
                            rhs=rn[:, t : t + 1],
                            start=(t == 0),
                            stop=(t == NSEG - 1),
                        )

            # gather the 4 bank columns into res; split DVE/ACT for overlap
            nc.vector.tensor_scalar_mul(res[:, 0:1], accs[0][:, 0:1], 1.0)
            nc.scalar.copy(out=res[:, 2:3], in_=accs[2][:, 0:1])
            nc.vector.tensor_scalar_mul(res[:, 1:2], accs[1][:, 0:1], 1.0)
            nc.scalar.copy(out=res[:, 3:4], in_=accs[3][:, 0:1])
            nc.sync.dma_start(out=s_out[:, :], in_=res)

    nc.finalize()
    return nc


def _shard(x: np.ndarray) -> list[dict]:
    xh = np.ascontiguousarray(x, dtype=np.float32).astype(np.float16)
    xh = xh.reshape(NCORES, P, NSEG, D)
    return [{"x": xh[c]} for c in range(NCORES)]


def _run(x: np.ndarray, trace: bool = False):
    global _NC
    if _NC is None:
        _NC = _build_nc()
    out = run_bass_kernel_spmd(
        _NC, _shard(x), core_ids=list(range(NCORES)), trace=trace
    )
    s = np.zeros(D, dtype=np.float64)
    for r in out.results:
        # res[p, c] = s[c*128 + p]
        s += r["s"].reshape(P, DCH).T.reshape(D).astype(np.float64)
    loss = (float(s @ s) - float(N)) / (N * TEMPERATURE)
    return np.asarray(loss, dtype=np.float32), out


def kernel(x: np.ndarray) -> np.ndarray:
    loss, _ = _run(x)
    return loss
